# revision 31
# baseline (speedup 1.0000x reference)
"""ArcFace loss (B=512, C=100000) on 8 TRN2 NeuronCores.

Vocab-parallel sharding: each core owns 12500 classes. Per core:
  - stream the (512, 12500) f32 shard; ScalarE computes exp(30*x - 30)
    with fused per-row accumulation (logits are in [0, 30), so a fixed
    stabilizer of 30 replaces the row max),
  - gather the 512 candidate target logits with indirect DMA, apply the
    additive angular margin via cos(t+m) = t*cos(m) - sin(m)*sqrt(1-t^2),
    and correct the local row-sum for the owned rows,
  - AllReduce (rowsum, target_logit) partials, then lse = 30 + ln(sum),
    nll = lse - s*margin_logit, mean over rows via a ones-vector matmul.
"""

import sys

import numpy as np

try:
    import concourse.bass as bass
except ImportError:  # pragma: no cover
    sys.path.insert(0, "/opt/trn_rl_repo")
    import concourse.bass as bass

import concourse.mybir as mybir
from concourse.bass_utils import run_bass_kernel_spmd

B = 512          # batch rows
C = 100000       # classes
NCORES = 8
CSH = C // NCORES  # classes per core: 12500
P = 128
G = B // P       # row groups: 4
NT = 2           # column tiles per row group
F = CSH // NT    # tile free dim: 6250
NTILES = G * NT  # 8
NBUF = 3         # streaming buffers
NWARM = 2        # EXP tiles before the margin-sqrt interleave

S = 30.0         # ArcFace scale
STAB = 30.0      # fixed logsumexp stabilizer (max possible logit)
CM = float(np.cos(0.5))
SM = float(np.sin(0.5))
CLIP_HI = float(np.float32(1.0 - 1e-7))
CLIP_LO = float(np.float32(-1.0 + 1e-7))

FP = mybir.dt.float32
I32 = mybir.dt.int32
AX = mybir.AxisListType
OP = mybir.AluOpType
AF = mybir.ActivationFunctionType


def build_nc():
    nc = bass.Bass()

    # register extra activation-bias constants (pattern from Bass.__init__)
    for val in (-STAB,):
        t = nc.alloc_sbuf_tensor(f"const-float32-{val}", [128, 1], FP)
        nc.gpsimd.memset(t.ap(), val)
        nc.const_aps.aps[(FP, val)] = t.ap()
    nc.all_engine_barrier()

    x = nc.declare_dram_parameter("x", [B * CSH], FP, isOutput=False)
    gofs = nc.declare_dram_parameter("gofs", [P, G], I32, isOutput=False)
    mask = nc.declare_dram_parameter("mask", [P, G], FP, isOutput=False)
    out_ext = nc.declare_dram_parameter("out", [1, 1], FP, isOutput=True)

    war_in = nc.dram_tensor("war_in", [P, 1], FP)
    war_out = nc.dram_tensor("war_out", [NCORES * P, 1], FP, addr_space="Shared")

    x2 = x.ap().rearrange("(r c) -> r c", c=CSH)
    xflat = x.ap().rearrange("(n o) -> n o", o=1)

    from contextlib import ExitStack
    with ExitStack() as ctx:
        sb = lambda name, shape, dt=FP: ctx.enter_context(
            nc.sbuf_tensor(name, shape, dt))
        xt = sb("xt", [P, NBUF * F])
        lnscr = sb("lnscr", [P, 1])
        acc = sb("acc", [P, NTILES])
        gofs_sb = sb("gofs_sb", [P, G], I32)
        mask_sb = sb("mask_sb", [P, G])
        t_sb = sb("t_sb", [P, G])
        tc = sb("tc", [P, G])
        t2 = sb("t2", [P, G])
        om = sb("om", [P, G])
        r = sb("r", [P, G])
        tcm = sb("tcm", [P, G])
        m = sb("m", [P, G])
        ms = sb("ms", [P, G])
        tl = sb("tl", [P, G])
        e1 = sb("e1", [P, G])
        e2 = sb("e2", [P, G])
        dd = sb("dd", [P, G])
        corr = sb("corr", [P, G])
        rs = sb("rs", [P, G])
        gbuf = sb("gbuf", [P, NCORES * 2 * G])  # slot 0 = own pack, 1..7 remote
        tot = sb("tot", [P, 2 * G])
        lg = sb("lg", [P, G])
        nll = sb("nll", [P, G])
        ones = sb("ones", [P, 1])
        res = sb("res", [1, 1])
        ps = ctx.enter_context(nc.psum_tensor("ps", [P, G], FP))
        dsems = [ctx.enter_context(nc.semaphore(f"dsem{b}"))
                 for b in range(NBUF)]
        psem = ctx.enter_context(nc.semaphore("psem"))
        gsem = ctx.enter_context(nc.semaphore("gsem"))
        vsem = ctx.enter_context(nc.semaphore("vsem"))
        ssem = ctx.enter_context(nc.semaphore("ssem"))
        csem = ctx.enter_context(nc.semaphore("csem"))
        wsem = ctx.enter_context(nc.semaphore("wsem"))
        msem = ctx.enter_context(nc.semaphore("msem"))
        prepsem = ctx.enter_context(nc.semaphore("prepsem"))
        lsem = ctx.enter_context(nc.semaphore("lsem"))
        rsem = ctx.enter_context(nc.semaphore("rsem"))
        block = ctx.enter_context(nc.Block())

        @block.sync
        def _(sync):
            zero_ap = nc.const_aps.aps[(FP, 0.0)]
            for j in range(NTILES):
                g, c = divmod(j, NT)
                if j >= NBUF:
                    sync.wait_ge(psem, j - NBUF + 1)
                b = j % NBUF
                sync.dma_start(
                    out=xt[:, b * F:(b + 1) * F],
                    in_=x2[g * P:(g + 1) * P, c * F:(c + 1) * F],
                ).then_inc(dsems[b], 16)
                if j == 2:
                    # init the warm-collective input (values unused) off the
                    # gpsimd critical path
                    sync.dma_start(out=war_in[:, :], in_=zero_ap).then_inc(
                        wsem, 16)
            # final scalar result out (HWDGE; sync is idle by now)
            sync.wait_ge(vsem, 6)
            sync.dma_start(out=out_ext[:1, :1], in_=res[:1, :1]).then_inc(
                dsems[0], 16)
            sync.wait_ge(dsems[0], 16 * (NTILES // NBUF + 1 + 1))

        @block.gpsimd
        def _(gpsimd):
            gpsimd.dma_start(out=gofs_sb[:, :], in_=gofs.ap()).then_inc(gsem, 16)
            gpsimd.dma_start(out=mask_sb[:, :], in_=mask.ap()).then_inc(gsem, 16)
            gpsimd.wait_ge(gsem, 32)
            for g in range(G):
                gpsimd.indirect_dma_start(
                    out=t_sb[:, g:g + 1],
                    out_offset=None,
                    in_=xflat,
                    in_offset=bass.IndirectOffsetOnAxis(
                        ap=gofs_sb[:, g:g + 1], axis=0
                    ),
                ).then_inc(gsem, 16)
            from concourse import library_config
            gpsimd.load_library(library_config.remote_dma)
            # all-gather of the (P, 8) stat pack via direct core-to-core DMA:
            # prep descriptors now (data is read at trigger time), fire later.
            # Sender s's slot-d broadcast lands at receiver r = s XOR d, so
            # slots hold a permutation of the 8 cores' packs — summed anyway.
            SL = 2 * G
            for dlt in range(1, NCORES):
                rd = [None] * NCORES
                rd[dlt] = (0, dlt)
                gpsimd.remote_dma_broadcast(
                    out_ap=gbuf[:, dlt * SL:(dlt + 1) * SL],
                    in_ap=gbuf[:, 0:SL],
                    remote_sem=rsem,
                    local_sem=lsem,
                    rdests=rd,
                ).then_inc(prepsem, 1)
            # warm-up collective: wakes ncfw and (critically) acts as the
            # entry fence proving every core started this execution before
            # any remote_dma send can land; result unused
            gpsimd.wait_ge(wsem, 16)
            gpsimd.collective_compute(
                "AllGather",
                mybir.AluOpType.bypass,
                replica_groups=[list(range(NCORES))],
                ins=[war_in[:, :]],
                outs=[war_out[:, :]],
            ).then_inc(csem, 1)
            gpsimd.wait_ge(prepsem, NCORES - 1)
            gpsimd.wait_ge(vsem, 3)   # own pack written into gbuf[:, 0:SL]
            gpsimd.wait_ge(csem, 1)   # every core is inside this execution
            gpsimd.trigger_dma(count=NCORES - 1)

        @block.vector
        def _(vector):
            vector.memset(ones[:, :], 1.0 / B)  # 1/B folded into matmul lhsT
            vector.drain()
            vector.wait_ge(gsem, 96)
            vector.tensor_scalar(tc[:, :], t_sb[:, :], CLIP_HI, CLIP_LO,
                                 op0=OP.min, op1=OP.max)
            vector.drain()
            vector.tensor_tensor(t2[:, :], tc[:, :], tc[:, :], op=OP.mult)
            vector.drain()
            vector.tensor_scalar(om[:, :], t2[:, :], -1.0, 1.0,
                                 op0=OP.mult, op1=OP.add).then_inc(vsem, 1)
            vector.wait_ge(ssem, 1)
            vector.tensor_scalar(tcm[:, :], tc[:, :], CM, None, op0=OP.mult)
            vector.drain()
            vector.scalar_tensor_tensor(m[:, :], in0=r[:, :], scalar=-SM,
                                        in1=tcm[:, :], op0=OP.mult, op1=OP.add)
            vector.drain()
            vector.tensor_scalar(ms[:, :], m[:, :], S, None,
                                 op0=OP.mult).then_inc(vsem, 1)
            vector.drain()
            vector.tensor_tensor(tl[:, :], ms[:, :], mask_sb[:, :], op=OP.mult)
            vector.wait_ge(ssem, 2)
            vector.tensor_tensor(dd[:, :], e2[:, :], e1[:, :], op=OP.subtract)
            vector.drain()
            vector.tensor_tensor(corr[:, :], dd[:, :], mask_sb[:, :], op=OP.mult)
            vector.drain()
            for g in range(G):
                vector.wait_ge(psem, NT * (g + 1))
                vector.tensor_reduce(rs[:, g:g + 1], acc[:, g * NT:(g + 1) * NT],
                                     axis=AX.X, op=OP.add)
            vector.drain()
            vector.tensor_tensor(gbuf[:, 0:G], rs[:, :], corr[:, :], op=OP.add)
            vector.drain()
            vector.tensor_copy(gbuf[:, G:2 * G], tl[:, :])
            vector.drain()
            vector.sem_inc(vsem, 1)   # own pack ready in gbuf[:, 0:2G]
            # remote packs land in slots 1..7; 2 rsem incs per arrival
            vector.wait_ge(rsem, 2 * (NCORES - 1))
            gview = gbuf.ap().rearrange("p (s c) -> p c s", c=2 * G)
            vector.tensor_reduce(tot[:, :], gview, axis=AX.X,
                                 op=OP.add).then_inc(vsem, 1)
            vector.wait_ge(ssem, 3)
            vector.scalar_tensor_tensor(nll[:, :], in0=lg[:, :], scalar=STAB,
                                        in1=tot[:, G:2 * G],
                                        op0=OP.add, op1=OP.subtract).then_inc(vsem, 1)
            vector.wait_ge(msem, 1)
            vector.tensor_reduce(res[:1, :1], ps[:1, :G], axis=AX.X,
                                 op=OP.add).then_inc(vsem, 1)

        @block.scalar
        def _(scalar):
            def exp_tile(j):
                b = j % NBUF
                scalar.wait_ge(dsems[b], 16 * (j // NBUF + 1))
                xs = xt[:, b * F:(b + 1) * F]
                scalar.activation(
                    xs, xs, AF.Exp,
                    bias=-STAB, scale=S,
                    accum_out=acc[:, j:j + 1],
                ).then_inc(psem, 1)

            # main pass starts immediately; margin math interleaves into the
            # DMA-bound gaps after NWARM tiles
            for j in range(NWARM):
                exp_tile(j)
            scalar.wait_ge(vsem, 1)
            scalar.activation(r[:, :], om[:, :], AF.Sqrt).then_inc(ssem, 1)
            scalar.wait_ge(vsem, 2)
            scalar.activation(e1[:, :], t_sb[:, :], AF.Exp, bias=-STAB, scale=S)
            scalar.activation(e2[:, :], ms[:, :], AF.Exp,
                              bias=-STAB, scale=1.0).then_inc(ssem, 1)
            for j in range(NWARM, NTILES):
                exp_tile(j)
            # dummy Ln to pull any table load off the post-gather path
            scalar.activation(lnscr[:, :], ones[:, :], AF.Ln)
            scalar.wait_ge(vsem, 4)
            scalar.activation(lg[:, :], tot[:, 0:G], AF.Ln).then_inc(ssem, 1)

        @block.tensor
        def _(tensor):
            tensor.wait_ge(vsem, 5)
            tensor.matmul(ps[:1, :G], lhsT=ones[:, :1], rhs=nll[:, :],
                          start=True, stop=True).then_inc(msem, 1)

    # populate .instr bytes for extended-ISA instructions (remote_dma etc.);
    # raw Bass skips this pass and the NEFF compiler then fails with
    # "ISA wrong length"
    from concourse.library_overlay import lower_extended_insts
    lower_extended_insts(nc)
    return nc


_CACHE = {}


def _get_nc():
    if "nc" not in _CACHE:
        _CACHE["nc"] = build_nc()
    return _CACHE["nc"]


def make_in_maps(x, label):
    x = np.asarray(x, dtype=np.float32)
    label = np.asarray(label).astype(np.int64)
    rows = np.arange(B, dtype=np.int64)
    in_maps = []
    for k in range(NCORES):
        off = k * CSH
        loc = label - off
        own = (loc >= 0) & (loc < CSH)
        locc = np.clip(loc, 0, CSH - 1)
        gofs_flat = (rows * CSH + locc).astype(np.int32)
        # partition layout: [p, g] holds row g*128 + p
        gofs_pg = np.ascontiguousarray(gofs_flat.reshape(G, P).T)
        mask_pg = np.ascontiguousarray(own.reshape(G, P).T.astype(np.float32))
        xs = np.ascontiguousarray(x[:, off:off + CSH]).reshape(-1)
        in_maps.append({"x": xs, "gofs": gofs_pg, "mask": mask_pg})
    return in_maps


def kernel(**inputs):
    nc = _get_nc()
    in_maps = make_in_maps(inputs["input"], inputs["label"])
    res = run_bass_kernel_spmd(nc, in_maps, core_ids=list(range(NCORES)))
    out = np.asarray(res.results[0]["out"], dtype=np.float32)
    return out.reshape(())
